# revision 1
# baseline (speedup 1.0000x reference)
"""Bidirectional Chamfer distance on 8 Trainium2 NeuronCores.

Problem: B=4 batches, N=M=4096 3-D points, f32.
  dist[b,n,m] = ||s[b,n]-t[b,m]||^2
  loss = mean_b( mean_n min_m dist + mean_m min_n dist )

Sharding: core c handles batch b=c//2, source-row half h=c%2
(2048 source rows x 4096 target cols per core).  Each core computes
  - rowmin (s2t direction) for its 2048 source rows (full min over m)
  - the column-min over its 2048 source rows (t2s direction), finished
    on-device with a gpsimd partition all-reduce; host combines the two
    half-cores per batch and averages.

Distance generation uses the TensorEngine: dist = saug^T @ taug with
augmented vectors (K=16, bf16 hi/lo split for fp32-exact products):
  a = 2*s:       a_hi, a_lo   (3+3+3+3 rows: all four hi/lo cross terms)
  -||s||^2:      ns_hi, ns_lo (2 rows, paired against 1)
  -||t||^2 side: nt_hi, nt_lo (2 rows, paired against -1)
  dot = (a_hi+a_lo).(t_hi+t_lo) - ||s||^2 - ||t||^2 = -dist exactly
All products of two bf16 are exact in the fp32 PSUM accumulation, so
the matmul matches fp32 up to PSUM accumulation order (~1e-7).

Everything runs in NEGATED-distance space (PE emits -dist) so all
reductions are max-based (required because gpsimd partition_all_reduce
only supports max, and it keeps the finale to a single op chain).

Per-core engine split (per 128-row stationary tile nt, 16 total, each
processed as two 2048-column halves):
  PE:   8 matmuls (128x512 each) -> PSUM (two 4-bank tiles in rotation)
  ACT:  drains PSUM -> SBUF fp16 half-copies (the sole PSUM reader;
        1 elem/cycle/lane @1.2GHz makes it the pipeline pacer)
  DVE:  row-max via tensor_scalar+accum_out(max) on the fp16 copy
        (4x perf mode), plus the column accumulator max chain
        (tensor_tensor fp16, 2x mode)
  Pool: final 128-partition all-reduce(max) of the column accumulator
fp16 staging adds < 1e-5 relative error (min/max never round, only the
single fp16 quantization of each distance matters, and it averages out).
CoreSim cost model: ~72 us per core (ACT 59, DVE 56, PE 28, Pool 7).
"""

import numpy as np
import ml_dtypes

B, N, M = 4, 4096, 4096
N_CORES = 8
NSH = N // 2          # 2048 source rows per core
K = 16                # augmented contraction dim
NT = NSH // 128       # 16 stationary tiles per core
MT = M // 512         # 8 moving chunks of 512

# (nt, half) pairs where DVE does a fused PSUM-read copy+rowmax instead
# of the ACT copy + 4x-mode DVE rowmax pair. Only (0, 0): it starts DVE
# ~3.5us earlier while ACT still waits for the first PSUM tile. (walrus
# only lowers add/mult tensor_tensor on Pool, so the whole max chain
# lives on DVE; Pool keeps the final partition reduce.)
FUSED_HALVES = frozenset(((0, 1), (4, 0)))
ACC_INIT = -60000.0  # fp16-safe "-infinity"; chains run on NEGATED dists (max)

_PROGRAM = None


def _build_program():
    import concourse.mybir as mybir
    import concourse.tile as tile
    from concourse import bacc, bass_isa
    from contextlib import ExitStack

    nc = bacc.Bacc(name="chamfer")
    f32 = mybir.dt.float32
    f16 = mybir.dt.float16
    bf16 = mybir.dt.bfloat16

    saugT = nc.dram_tensor("saugT", [K, NSH], bf16, kind="ExternalInput")
    taugT = nc.dram_tensor("taugT", [K, M], bf16, kind="ExternalInput")
    # out_s2t[p, 2*nt+half] = max over the half's m of -dist for source
    # row nt*128+p (host combines the two halves per nt)
    out_s2t = nc.dram_tensor("out_s2t", [128, 2 * NT], f32, kind="ExternalOutput")
    # out_t2s[j] = -min over this core's source rows of dist[:, j] (fp16)
    out_t2s = nc.dram_tensor("out_t2s", [1, M], f16, kind="ExternalOutput")

    with tile.TileContext(nc) as tc, ExitStack() as ctx:
        inputs = ctx.enter_context(tc.tile_pool(name="inputs", bufs=1))
        psum_pool = ctx.enter_context(tc.tile_pool(name="psum", bufs=2, space="PSUM"))
        copy_pool = ctx.enter_context(tc.tile_pool(name="copies", bufs=8))
        accp = ctx.enter_context(tc.tile_pool(name="accp", bufs=1))
        outp = ctx.enter_context(tc.tile_pool(name="outp", bufs=1))

        saug = inputs.tile([K, NSH], bf16)
        for i in range(2):
            nc.sync.dma_start(
                out=saug[:, i * (NSH // 2):(i + 1) * (NSH // 2)],
                in_=saugT[:, i * (NSH // 2):(i + 1) * (NSH // 2)],
            )
        taug = inputs.tile([K, M], bf16)
        for i in range(4):
            nc.gpsimd.dma_start(
                out=taug[:, i * (M // 4):(i + 1) * (M // 4)],
                in_=taugT[:, i * (M // 4):(i + 1) * (M // 4)],
            )

        accA = accp.tile([128, M], f16)   # DVE chain accumulator
        accB = accp.tile([128, M], f16)   # partition-reduce output
        nc.gpsimd.memset(accA, ACC_INIT)

        # rowmin partials: two slots per nt (fused path writes one per half)
        rowpart = outp.tile([128, 2 * NT], f32)
        nc.vector.memset(rowpart, -3.0e38)

        for nt in range(NT):
            for half in range(2):
                ps = psum_pool.tile([128, 2048], f32, tag="ps")
                for q in range(4):
                    mt = half * 4 + q
                    nc.tensor.matmul(
                        ps[:, q * 512:(q + 1) * 512],
                        saug[:, nt * 128:(nt + 1) * 128],
                        taug[:, mt * 512:(mt + 1) * 512],
                        start=True,
                        stop=True,
                    )
                cph = copy_pool.tile([128, 2048], f16, tag="cph")
                slot = rowpart[:, 2 * nt + half:2 * nt + half + 1]
                if (nt, half) in FUSED_HALVES:
                    # DVE: fused PSUM->SBUF fp16 copy + rowmin partial
                    nc.vector.tensor_scalar(
                        out=cph,
                        in0=ps,
                        scalar1=0.0,
                        scalar2=None,
                        op0=mybir.AluOpType.add,
                        op1=mybir.AluOpType.max,
                        accum_out=slot,
                    )
                else:
                    # ScalarE stages PSUM -> SBUF (casting to fp16)
                    nc.scalar.copy(out=cph, in_=ps)

                # column accumulator half-chain (DVE, fp16 2x mode);
                # issued before the rowmax so cph/psum slots release sooner
                acch = accA[:, half * 2048:(half + 1) * 2048]
                nc.vector.tensor_tensor(
                    out=acch, in0=cph, in1=acch, op=mybir.AluOpType.max
                )
                if (nt, half) not in FUSED_HALVES:
                    # rowmax over the fp16 half-copy (4x DVE mode)
                    nc.vector.tensor_scalar(
                        out=cph,
                        in0=cph,
                        scalar1=0.0,
                        scalar2=None,
                        op0=mybir.AluOpType.add,
                        op1=mybir.AluOpType.max,
                        accum_out=slot,
                    )

        nc.sync.dma_start(out=out_s2t[:, :], in_=rowpart)

        # partition-reduce (negated space: max = -min), split into quarters
        # so Pool reduces overlap the tail of the DVE chain
        for h in range(4):
            sl = slice(h * (M // 4), (h + 1) * (M // 4))
            nc.gpsimd.partition_all_reduce(
                accB[:, sl], accA[:, sl], 128, bass_isa.ReduceOp.max
            )
            nc.sync.dma_start(out=out_t2s[:, sl], in_=accB[0:1, sl])

    nc.finalize()
    return nc


def _augment(source, target):
    """Per-core augmented bf16 hi/lo operands."""
    bf = ml_dtypes.bfloat16

    def split(x):
        hi = x.astype(bf)
        lo = (x - hi.astype(np.float32)).astype(bf)
        return hi, lo

    in_maps = []
    for c in range(N_CORES):
        b, h = c // 2, c % 2
        s = np.asarray(source[b, h * NSH:(h + 1) * NSH], dtype=np.float32)  # (NSH,3)
        t = np.asarray(target[b], dtype=np.float32)                         # (M,3)
        # NEGATED-distance space: dot(saug, taug) = -dist
        a = 2.0 * s
        ns = -(s * s).sum(axis=1, dtype=np.float32)
        ntg = (t * t).sum(axis=1, dtype=np.float32)
        ah, al = split(a)
        th, tl = split(t)
        nsh_, nsl = split(ns)
        nth, ntl = split(ntg)
        ones_s = np.ones(NSH, dtype=bf)
        ones_t = np.ones(M, dtype=bf)

        saugT = np.empty((K, NSH), dtype=bf)
        taugT = np.empty((K, M), dtype=bf)
        saugT[0:3] = ah.T
        taugT[0:3] = th.T
        saugT[3:6] = ah.T
        taugT[3:6] = tl.T
        saugT[6:9] = al.T
        taugT[6:9] = th.T
        saugT[9:12] = al.T
        taugT[9:12] = tl.T
        saugT[12] = nsh_
        saugT[13] = nsl
        taugT[12] = ones_t
        taugT[13] = ones_t
        saugT[14] = -ones_s
        saugT[15] = -ones_s
        taugT[14] = nth
        taugT[15] = ntl

        in_maps.append({"saugT": saugT, "taugT": taugT})
    return in_maps


# test harness hook: set _BENCH["trace"]=True to profile; results land in
# _BENCH["last"] (BassKernelResults with exec_time_ns).
_BENCH = {"trace": False, "last": None}


def kernel(source, target):
    global _PROGRAM
    from concourse.bass_utils import run_bass_kernel_spmd

    source = np.asarray(source, dtype=np.float32)
    target = np.asarray(target, dtype=np.float32)

    if _PROGRAM is None:
        _PROGRAM = _build_program()

    in_maps = _augment(source, target)
    bkr = run_bass_kernel_spmd(
        _PROGRAM, in_maps, list(range(N_CORES)), trace=_BENCH["trace"]
    )
    _BENCH["last"] = bkr
    res = bkr.results

    # device values are in negated-distance space (maxima of -dist)
    loss = np.float64(0.0)
    for b in range(B):
        r0, r1 = res[2 * b], res[2 * b + 1]
        def rowmax_neg(r):
            rp = r["out_s2t"]                     # (128, 2*NT) half partials
            return np.maximum(rp[:, 0::2], rp[:, 1::2]).T.reshape(-1)
        rowmin = -np.concatenate([rowmax_neg(r0), rowmax_neg(r1)])  # (N,)
        colmin = -np.maximum(
            r0["out_t2s"][0].astype(np.float32), r1["out_t2s"][0].astype(np.float32)
        )  # (M,)
        loss += rowmin.mean(dtype=np.float64) + colmin.mean(dtype=np.float64)
    return np.float32(loss / B)



# revision 2
# speedup vs baseline: 1.0526x; 1.0526x over previous
"""Bidirectional Chamfer distance on 8 Trainium2 NeuronCores — v2.

Problem: B=4 batches, N=M=4096 3-D points, f32.
  dist[b,n,m] = ||s[b,n]-t[b,m]||^2
  loss = mean_b( mean_n min_m dist + mean_m min_n dist )

Sharding: core c handles batch b=c//2, source-row half h=c%2
(2048 source rows x 4096 target cols per core).

Distance generation: TensorEngine, dist = saug^T @ taug with augmented
K=16 bf16 hi/lo vectors (fp32-exact); PE emits NEGATED distance so all
reductions are max-based (gpsimd partition_all_reduce supports max
only).

v2 pipeline (v1 was ACT-bound at ~59us with Pool idle):
  - PSUM is a 4-slot conveyor of [128,1024] strips (2 banks each), so
    the two extraction engines (ACT copy / DVE fused copy+rowmax) run
    concurrently on different strips and a slow strip never stalls PE.
  - Per strip, extraction is ACT copy -> d16 fp16 (rows later via DVE
    tensor_scalar @4x on the fp16 strip) or DVE tensor_scalar from
    PSUM (copy + row partial in one 1x op).
  - Per nt tile [128,4096] of d16, the column reduction is either a
    DVE chain (tt fp16 2x into acc) or a Pool partition_all_reduce
    whose row 0 goes straight to DRAM (host min-combines). Chain tiles
    run early so the acc finals hide mid-stream; the last pool tiles
    reduce per-strip to shorten the tail.
"""

import numpy as np
import ml_dtypes

B, N, M = 4, 4096, 4096
N_CORES = 8
NSH = N // 2          # 2048 source rows per core
K = 16                # augmented contraction dim
NT = NSH // 128       # 16 stationary tiles per core
NS = 4                # strips per tile
W = M // NS           # strip width (1024)

CFG = {
    # (nt, strip) extracted by DVE fused op; rest ACT
    "dve_strips": tuple(
        (i // NS, i % NS) for i in range(4, NT * NS) if (i - 4) % 3 == 0
    )[:19],
    # tiles whose column reduction runs on Pool (rest: DVE chain).
    # Tile 15 must chain so the acc finals pipeline per-strip at the end;
    # no pool tiles after 13 so Pool is free for the finals.
    "pool_tiles": (1, 2, 3, 5, 6, 7, 9, 10, 11, 12, 13),
    # pool tiles (by position from the end) reduced per-strip
    "pool_striped_tail": 0,
    # reduction-issue lag in strips
    "defer": 2,
}

_PROGRAM = None


def _build_program(cfg=CFG):
    import concourse.mybir as mybir
    import concourse.tile as tile
    from concourse import bacc, bass_isa
    from contextlib import ExitStack

    dve_strips = frozenset(map(tuple, cfg["dve_strips"]))
    pool_tiles = tuple(cfg["pool_tiles"])
    pool_set = frozenset(pool_tiles)
    defer = cfg["defer"]
    striped = frozenset(pool_tiles[len(pool_tiles) - cfg["pool_striped_tail"]:])
    chain_tiles = [nt for nt in range(NT) if nt not in pool_set]
    first_chain = chain_tiles[0] if chain_tiles else None
    last_chain = chain_tiles[-1] if chain_tiles else None

    nc = bacc.Bacc(name="chamfer2")
    f32 = mybir.dt.float32
    f16 = mybir.dt.float16
    bf16 = mybir.dt.bfloat16

    saugT = nc.dram_tensor("saugT", [K, NSH], bf16, kind="ExternalInput")
    taugT = nc.dram_tensor("taugT", [K, M], bf16, kind="ExternalInput")
    # row partials per (nt, strip); host maxes groups of NS (negated space)
    out_s2t = nc.dram_tensor("out_s2t", [128, NT * NS], f32, kind="ExternalOutput")
    # chain acc after final partition reduce (row 0 valid)
    out_t2s = nc.dram_tensor("out_t2s", [1, M], f16, kind="ExternalOutput")
    # Pool-reduced column partials, one row per pool tile
    out_t2s_pool = nc.dram_tensor(
        "out_t2s_pool", [max(len(pool_tiles), 1), M], f16, kind="ExternalOutput"
    )

    with tile.TileContext(nc) as tc, ExitStack() as ctx:
        inputs = ctx.enter_context(tc.tile_pool(name="inputs", bufs=1))
        psum_pool = ctx.enter_context(tc.tile_pool(name="psum", bufs=4, space="PSUM"))
        d16_pool = ctx.enter_context(tc.tile_pool(name="d16", bufs=6))
        red_pool = ctx.enter_context(tc.tile_pool(name="red", bufs=3))
        accp = ctx.enter_context(tc.tile_pool(name="accp", bufs=1))
        outp = ctx.enter_context(tc.tile_pool(name="outp", bufs=1))

        saug = inputs.tile([K, NSH], bf16)
        taug = inputs.tile([K, M], bf16)
        # tiny first chunks so the first matmul (saug[:,0:128] x
        # taug[:,0:512]) can start ~2.5us earlier; bulk follows
        nc.sync.dma_start(out=saug[:, 0:128], in_=saugT[:, 0:128])
        nc.sync.dma_start(out=taug[:, 0:512], in_=taugT[:, 0:512])
        nc.sync.dma_start(out=taug[:, 512:1024], in_=taugT[:, 512:1024])
        nc.sync.dma_start(out=taug[:, 1024:2048], in_=taugT[:, 1024:2048])
        nc.sync.dma_start(out=taug[:, 2048:3072], in_=taugT[:, 2048:3072])
        nc.sync.dma_start(out=taug[:, 3072:4096], in_=taugT[:, 3072:4096])
        nc.sync.dma_start(out=saug[:, 128:NSH // 2], in_=saugT[:, 128:NSH // 2])
        nc.sync.dma_start(
            out=saug[:, NSH // 2:NSH], in_=saugT[:, NSH // 2:NSH]
        )

        acc = accp.tile([128, M], f16)      # DVE column-chain accumulator
        junk = accp.tile([128, 2 * W], f16)  # rows-op elementwise sink
        rowpart = outp.tile([128, NT * NS], f32)
        nc.vector.memset(rowpart, -3.0e38)

        d16s = {}

        def issue_reduction(nt, s):
            """rows (if ACT strip) + column-reduction share of strip s."""
            d16 = d16s[nt]
            ds = d16[:, s * W:(s + 1) * W]
            if (nt, s) not in dve_strips:
                # pair two adjacent ACT strips into one 2048-wide 4x rows op
                pair_next = s % 2 == 0 and (nt, s + 1) not in dve_strips
                pair_prev = s % 2 == 1 and (nt, s - 1) not in dve_strips
                if not pair_next:
                    lo = s - 1 if pair_prev else s
                    slot = rowpart[:, nt * NS + lo:nt * NS + lo + 1]
                    wid = (s + 1 - lo) * W
                    nc.vector.tensor_scalar(
                        out=junk[:, 0:wid], in0=d16[:, lo * W:(s + 1) * W],
                        scalar1=0.0, scalar2=None,
                        op0=mybir.AluOpType.add, op1=mybir.AluOpType.max,
                        accum_out=slot,
                    )

            if nt in pool_set:
                if nt in striped:
                    red = red_pool.tile([128, W], f16, tag="redq")
                    nc.gpsimd.partition_all_reduce(
                        red, ds, 128, bass_isa.ReduceOp.max
                    )
                    i = pool_tiles.index(nt)
                    nc.sync.dma_start(
                        out=out_t2s_pool[i:i + 1, s * W:(s + 1) * W],
                        in_=red[0:1, :],
                    )
                elif s == NS - 1:
                    red = red_pool.tile([128, M], f16, tag="redf")
                    nc.gpsimd.partition_all_reduce(
                        red, d16, 128, bass_isa.ReduceOp.max
                    )
                    i = pool_tiles.index(nt)
                    nc.sync.dma_start(
                        out=out_t2s_pool[i:i + 1, :], in_=red[0:1, :]
                    )
            else:
                accs = acc[:, s * W:(s + 1) * W]
                if nt == first_chain:
                    nc.vector.tensor_scalar_add(out=accs, in0=ds, scalar1=0.0)
                else:
                    nc.vector.tensor_tensor(
                        out=accs, in0=ds, in1=accs, op=mybir.AluOpType.max
                    )
                if nt == last_chain:
                    # acc final for this quarter-column range, hidden
                    # mid-stream; row 0 -> DRAM
                    red = red_pool.tile([128, W], f16, tag="redq")
                    nc.gpsimd.partition_all_reduce(
                        red, accs, 128, bass_isa.ReduceOp.max
                    )
                    nc.sync.dma_start(
                        out=out_t2s[:, s * W:(s + 1) * W],
                        in_=red[0:1, :],
                    )

        seq = [(nt, s) for nt in range(NT) for s in range(NS)]
        for idx, (nt, s) in enumerate(seq):
            if s == 0:
                d16s[nt] = d16_pool.tile([128, M], f16, tag="d16", name=f"d16_{nt}")
            ps = psum_pool.tile([128, W], f32, tag="ps")
            for q in range(2):
                c0 = s * W + q * 512
                nc.tensor.matmul(
                    ps[:, q * 512:(q + 1) * 512],
                    saug[:, nt * 128:(nt + 1) * 128],
                    taug[:, c0:c0 + 512],
                    start=True,
                    stop=True,
                )
            ds = d16s[nt][:, s * W:(s + 1) * W]
            if (nt, s) in dve_strips:
                slot = rowpart[:, nt * NS + s:nt * NS + s + 1]
                nc.vector.tensor_scalar(
                    out=ds, in0=ps, scalar1=0.0, scalar2=None,
                    op0=mybir.AluOpType.add, op1=mybir.AluOpType.max,
                    accum_out=slot,
                )
            else:
                nc.scalar.copy(out=ds, in_=ps)

            if idx >= defer:
                issue_reduction(*seq[idx - defer])

        for idx in range(len(seq) - defer, len(seq)):
            issue_reduction(*seq[idx])

        nc.sync.dma_start(out=out_s2t[:, :], in_=rowpart)

    nc.finalize()
    return nc


def _augment(source, target):
    """Per-core augmented bf16 hi/lo operands (negated-distance space)."""
    bf = ml_dtypes.bfloat16

    def split(x):
        hi = x.astype(bf)
        lo = (x - hi.astype(np.float32)).astype(bf)
        return hi, lo

    in_maps = []
    for c in range(N_CORES):
        b, h = c // 2, c % 2
        s = np.asarray(source[b, h * NSH:(h + 1) * NSH], dtype=np.float32)
        t = np.asarray(target[b], dtype=np.float32)
        a = 2.0 * s
        ns = -(s * s).sum(axis=1, dtype=np.float32)
        ntg = (t * t).sum(axis=1, dtype=np.float32)
        ah, al = split(a)
        th, tl = split(t)
        nsh_, nsl = split(ns)
        nth, ntl = split(ntg)
        ones_s = np.ones(NSH, dtype=bf)
        ones_t = np.ones(M, dtype=bf)

        saugT = np.empty((K, NSH), dtype=bf)
        taugT = np.empty((K, M), dtype=bf)
        saugT[0:3] = ah.T
        taugT[0:3] = th.T
        saugT[3:6] = ah.T
        taugT[3:6] = tl.T
        saugT[6:9] = al.T
        taugT[6:9] = th.T
        saugT[9:12] = al.T
        taugT[9:12] = tl.T
        saugT[12] = nsh_
        saugT[13] = nsl
        taugT[12] = ones_t
        taugT[13] = ones_t
        saugT[14] = -ones_s
        saugT[15] = -ones_s
        taugT[14] = nth
        taugT[15] = ntl

        in_maps.append({"saugT": saugT, "taugT": taugT})
    return in_maps


_BENCH = {"trace": False, "last": None}


def kernel(source, target):
    global _PROGRAM
    from concourse.bass_utils import run_bass_kernel_spmd

    source = np.asarray(source, dtype=np.float32)
    target = np.asarray(target, dtype=np.float32)

    if _PROGRAM is None:
        _PROGRAM = _build_program()

    in_maps = _augment(source, target)
    bkr = run_bass_kernel_spmd(
        _PROGRAM, in_maps, list(range(N_CORES)), trace=_BENCH["trace"]
    )
    _BENCH["last"] = bkr
    res = bkr.results

    loss = np.float64(0.0)
    for b in range(B):
        r0, r1 = res[2 * b], res[2 * b + 1]

        def rowmin_core(r):
            rp = r["out_s2t"]  # (128, NT*NS)
            return -(rp.reshape(128, NT, NS).max(axis=2).T.reshape(-1))

        def colneg_core(r):
            cm = r["out_t2s"][0].astype(np.float32)
            pool_rows = r["out_t2s_pool"].astype(np.float32)
            return np.maximum(cm, pool_rows.max(axis=0))

        rowmin = np.concatenate([rowmin_core(r0), rowmin_core(r1)])  # (N,)
        colmin = -np.maximum(colneg_core(r0), colneg_core(r1))       # (M,)
        loss += rowmin.mean(dtype=np.float64) + colmin.mean(dtype=np.float64)
    return np.float32(loss / B)
